# revision 21
# baseline (speedup 1.0000x reference)
"""TRN2 Bass/Tile kernel: 16-head MHA, B=1 S=4096 E=1024, head-sharded over 8 cores.

Sharding: tensor-parallel over heads. Core c owns heads {2c, 2c+1}: columns
[128c, 128(c+1)) of Wq/Wk/Wv (+bias slices) and rows [128c, 128(c+1)) of Wo.
Each core computes attention for its 2 heads and a partial out-projection
[S, E] (fp16); the host sums the 8 partials and adds bo.

Per-core pipeline (fp16 projections, fp8e4m3 DoubleRow PV, fp32 PSUM):
  Prefix) KT/VT [128ch, S] = W^T @ x (xT resident in SBUF), V repacked
          natural [k, ch] via PE transpose into fp8 V2 = [V_h0|1 .. V_h1|1].
  Loop over 8 q-blocks of 512:
    per key-tile kt (128 keys): scores^T [k, q] via TWO row-tiled matmuls
    (h0 on PE rows 0:63, h1 on rows 64:127) -> [128, 1024] psum slot;
    exp -> fp8: even kt on ACT (native Exp), odd kt on DVE (Schraudolph:
    int8(a*s+b) bitcast as fp8e4m3 ~= exp(s/8)); PV accumulates kt-PAIRS
    with fp8 DoubleRow matmuls, psum[65, 512]/head (row 64 = denominator).
    Normalize via DMA-spread recip; out-proj + next Q-proj are deferred
    into the next q-block's PE stream to keep all engines pipelined.
"""

import sys

for _p in ("/opt/trn_rl_repo", "/opt/pypackages"):
    if _p not in sys.path:
        sys.path.append(_p)

import numpy as np

EMBED = 1024
N_CORES = 8
HC = EMBED // N_CORES  # 128 channels = 2 heads per core
DH = 64                # head dim
SEQ = 4096

_NC_CACHE = {}

# fp16 Schraudolph (unused when DR enabled, kept for fallback):
SCH16_A = 1024.0 / np.log(2.0) * 0.125
SCH16_B = 15360.0 - 61.0 + 0.5
# fp8e4m3 Schraudolph: exp(0.125*s) ~= bitcast_fp8(int8(A*s + B))
SCH8_A = 8.0 / np.log(2.0) * 0.125
SCH8_B = 56.0 - 0.477 + 0.5


def _build_nc(S=SEQ, E=EMBED, mmdt="fp16"):
    from contextlib import ExitStack

    import concourse.bass as bass
    import concourse.mybir as mybir
    import concourse.tile as tile
    from concourse import bacc
    from concourse.masks import make_identity

    assert mmdt == "fp16", "only fp16 matmul path implemented"
    F32 = mybir.dt.float32
    F16 = mybir.dt.float16
    F8 = mybir.dt.float8e4
    I8 = mybir.dt.int8
    DR = mybir.MatmulPerfMode.DoubleRow

    ET = E // 128      # 8 contraction tiles for projections
    NSC = S // 512     # 8 S-chunks of 512
    NKT = S // 128     # 32 key tiles of 128
    NPR = NKT // 2     # 16 kt pairs
    NQS = 512 // 128   # q subtiles per block
    NEC = E // 512     # out-proj 512-wide chunks

    nc = bacc.Bacc()
    xT = nc.declare_dram_parameter("xT", [E, S], F16, isOutput=False)
    wq = nc.declare_dram_parameter("wq", [128, ET * HC], F16, isOutput=False)
    wk = nc.declare_dram_parameter("wk", [128, ET * HC], F16, isOutput=False)
    wv = nc.declare_dram_parameter("wv", [128, ET * HC], F16, isOutput=False)
    bq = nc.declare_dram_parameter("bq", [HC, 1], F32, isOutput=False)
    bk = nc.declare_dram_parameter("bk", [HC, 1], F32, isOutput=False)
    bv = nc.declare_dram_parameter("bv", [HC, 1], F32, isOutput=False)
    wo = nc.declare_dram_parameter("wo", [HC, E], F16, isOutput=False)
    out = nc.declare_dram_parameter("out", [S, E], F16, isOutput=True)

    with tile.TileContext(nc) as tc, ExitStack() as ctx:
        wpool = ctx.enter_context(tc.tile_pool(name="w", bufs=1))
        xpool = ctx.enter_context(tc.tile_pool(name="x", bufs=1))
        kvpool = ctx.enter_context(tc.tile_pool(name="kv", bufs=1))
        qpool = ctx.enter_context(tc.tile_pool(name="q", bufs=2))
        expool = ctx.enter_context(tc.tile_pool(name="e", bufs=5))
        apool = ctx.enter_context(tc.tile_pool(name="a", bufs=2))
        rpool = ctx.enter_context(tc.tile_pool(name="r", bufs=2))
        opool = ctx.enter_context(tc.tile_pool(name="o", bufs=2))
        dpool = ctx.enter_context(tc.tile_pool(name="d", bufs=2, space="DRAM"))
        # PSUM: 2x [128,1024] score slots + 2 PV accum + [128,1024] op = 8
        spsum = ctx.enter_context(tc.tile_pool(name="sp", bufs=2, space="PSUM"))
        pvpsum = ctx.enter_context(tc.tile_pool(name="pv", bufs=1, space="PSUM"))
        opsum = ctx.enter_context(tc.tile_pool(name="op", bufs=1, space="PSUM"))

        # --- weight/bias DMAs for the prefix first, then x, then the rest ---
        w_sb = {}
        for name, src in (("wk", wk), ("wv", wv)):
            t = wpool.tile([128, ET, HC], F16, tag=name, name=name)
            nc.sync.dma_start(out=t, in_=src.rearrange("p (a c) -> p a c", c=HC))
            w_sb[name] = t
        b_sb = {}
        for name, src in (("bk", bk), ("bv", bv)):
            t = wpool.tile([HC, 1], F32, tag=name, name=name)
            nc.sync.dma_start(out=t, in_=src[:, :])
            b_sb[name] = t
        xfull = xpool.tile([128, ET, S], F16, tag="xf")
        for cc in range(S // 1024):
            for et in range(ET):
                nc.sync.dma_start(
                    out=xfull[:, et, cc * 1024:(cc + 1) * 1024],
                    in_=xT[et * 128:(et + 1) * 128, cc * 1024:(cc + 1) * 1024],
                )
        t = wpool.tile([128, ET, HC], F16, tag="wq", name="wq")
        nc.sync.dma_start(out=t, in_=wq.rearrange("p (a c) -> p a c", c=HC))
        w_sb["wq"] = t
        t = wpool.tile([HC, 1], F32, tag="bq", name="bq")
        nc.sync.dma_start(out=t, in_=bq[:, :])
        b_sb["bq"] = t
        wo_sb = wpool.tile([HC, E], F16, tag="wo")
        nc.sync.dma_start(out=wo_sb, in_=wo[:, :])

        ones64 = wpool.tile([1, 64], F32, tag="ones64")
        nc.vector.memset(ones64, 1.0)
        ident = wpool.tile([128, 128], F16, tag="ident")
        make_identity(nc, ident)
        # tiny dummy exp so the ACT exp table set loads during the DMA prefix
        dummy = wpool.tile([1, 2], F32, tag="dummy")
        nc.vector.memset(dummy[:, 0:1], 0.0)
        nc.scalar.activation(dummy[:, 1:2], dummy[:, 0:1],
                             mybir.ActivationFunctionType.Exp)

        # --- prefix: KT [128ch, S]; V2 fp8 [128k, NKT, 160] ---
        # V2[:, kt, h, 0:65] = [V_h | 1] (fp8, 80-col stride for DR APs)
        KT = kvpool.tile([128, S], F16, tag="KT")
        V2 = kvpool.tile([128, NKT, 2, 80], F8, tag="V2")
        nc.vector.memset(V2[:, :, :, 64:65], 1.0)
        for scp in range(NSC // 2):
            csl = slice(scp * 1024, (scp + 1) * 1024)
            pk = spsum.tile([128, 1024], F32, tag="s")
            pv_ = spsum.tile([128, 1024], F32, tag="s")
            for et in range(ET):
                first, last = et == 0, et == ET - 1
                for h2 in (0, 1):
                    xs = xfull[:, et, scp * 1024 + h2 * 512:
                               scp * 1024 + (h2 + 1) * 512]
                    nc.tensor.matmul(pk[:, h2 * 512:(h2 + 1) * 512],
                                     lhsT=w_sb["wk"][:, et, :], rhs=xs,
                                     start=first, stop=last)
                for h2 in (0, 1):
                    xs = xfull[:, et, scp * 1024 + h2 * 512:
                               scp * 1024 + (h2 + 1) * 512]
                    nc.tensor.matmul(pv_[:, h2 * 512:(h2 + 1) * 512],
                                     lhsT=w_sb["wv"][:, et, :], rhs=xs,
                                     start=first, stop=last)
            nc.vector.tensor_scalar_add(KT[:, csl], pk, b_sb["bk"])
            VTp = qpool.tile([128, 1024], F16, tag="vt")
            nc.vector.tensor_scalar_add(VTp, pv_, b_sb["bv"])
            for j in range(8):
                kt = scp * 8 + j
                pt = pvpsum.tile([128, 512], F16,
                                 tag="pv0" if j % 2 == 0 else "pv1")
                nc.tensor.transpose(pt[:, 0:128],
                                    VTp[:, j * 128:(j + 1) * 128], ident)
                nc.vector.tensor_copy(
                    V2[:, kt, :, 0:64],
                    pt[:, 0:128].rearrange("p (a c) -> p a c", a=2))

        # --- main loop over q-blocks ---
        # All cross-block work (normalization chain, out-proj, next Q-proj)
        # is staged as step-scheduled emissions inside the NEXT q-block's
        # kt loop, spaced so each engine's strict-FIFO queue never blocks
        # on an input that is not ready yet.
        from concourse.tile import add_dep_helper
        fence = [None]  # most recent scores MM, to order deferred PE work
        dr_fence = [None]  # last DR matmul, to pin batch order on the PE
        sched = {}      # step -> [emitters] for the current q-block's loop

        def emit_qproj(qb):
            qsl = slice(qb * 512, (qb + 1) * 512)
            pq = opsum.tile([128, 1024], F32, tag="op2")
            for et in range(ET):
                mm = nc.tensor.matmul(pq[:, 0:512], lhsT=w_sb["wq"][:, et, :],
                                      rhs=xfull[:, et, qsl],
                                      start=et == 0, stop=et == ET - 1)
                if fence[0] is not None:
                    add_dep_helper(mm.ins, fence[0].ins, sync=False,
                                   reason="defer qproj")
            QTb = qpool.tile([128, 512], F16, tag="qt")
            nc.vector.tensor_scalar_add(QTb, pq[:, 0:512], b_sb["bq"])
            return QTb

        QTbs = {0: emit_qproj(0)}

        def reg(step, fn):
            sched.setdefault(step, []).append(fn)

        def make_qp(nqb):
            def emit():
                QTbs[nqb] = emit_qproj(nqb)
            return emit

        if NSC > 1:
            reg(10, make_qp(1))

        for qb in range(NSC):
            QTb = QTbs.pop(qb)
            pv0 = pvpsum.tile([128, 512], F32, tag="pv0")
            pv1 = pvpsum.tile([128, 512], F32, tag="pv1")
            expair = None
            expairs = {}
            nsched, sched = sched, {}
            for kt in range(NKT):
                for fn in nsched.pop(kt, ()):
                    fn()
                ksl = slice(kt * 128, (kt + 1) * 128)
                slot = spsum.tile([128, 1024], F32, tag="s")
                fence[0] = mm0 = nc.tensor.matmul(
                    slot[:, 0:512], lhsT=KT[0:64, ksl],
                    rhs=QTb[0:64, :], start=True, stop=True)
                mm1 = nc.tensor.matmul(slot[:, 512:1024], lhsT=KT[64:128, ksl],
                                       rhs=QTb[64:128, :],
                                       start=True, stop=True)
                if dr_fence[0] is not None:
                    # keep scores runs contiguous: schedule them after the
                    # last emitted DR batch, never inside it
                    add_dep_helper(mm0.ins, dr_fence[0].ins, sync=False,
                                   reason="scores after DR batch")
                    add_dep_helper(mm1.ins, dr_fence[0].ins, sync=False,
                                   reason="scores after DR batch")
                if kt % 2 == 0:
                    expair = expool.tile([128, 2, 1024], F8, tag="ex")
                if kt % 2 == 0 or kt < 8:
                    # ACT native exp: evens + {1,3,5,7} = 20 of 32 kts (ACT is
                    # the faster engine; DVE carries the osb/misc copies)
                    nc.scalar.activation(
                        expair[:, kt % 2, :], slot,
                        mybir.ActivationFunctionType.Exp, scale=0.125)
                else:
                    nc.vector.tensor_scalar(
                        expair.bitcast(I8)[:, 1, :], slot, SCH8_A, SCH8_B,
                        op0=mybir.AluOpType.mult, op1=mybir.AluOpType.add)
                if kt % 2 == 1:
                    expairs[kt // 2] = expair
                if kt % 4 == 3 and kt >= 7:
                    # batch DR PV matmuls for 2 expair-pairs, lagged 2 pairs
                    # behind the exps: fewer normal<->DoubleRow mode switches
                    # (each costs an array drain), scores stay adjacent for
                    # tile concurrency, and the exps have ~4 kts of slack
                    for t in (kt // 2 - 3, kt // 2 - 2):
                        ex_t = expairs.pop(t)
                        d0 = nc.tensor.matmul(
                            pv0[0:65, :], lhsT=V2[:, 2 * t:2 * t + 2, 0, 0:65],
                            rhs=ex_t[:, :, 0:512], perf_mode=DR,
                            start=t == 0, stop=t == NPR - 1)
                        add_dep_helper(d0.ins, fence[0].ins, sync=False,
                                       reason="DR batch after scores")
                        dr_fence[0] = nc.tensor.matmul(
                            pv1[0:65, :], lhsT=V2[:, 2 * t:2 * t + 2, 1, 0:65],
                            rhs=ex_t[:, :, 512:1024], perf_mode=DR,
                            start=t == 0, stop=t == NPR - 1)
            # leftover DR pairs (exps already done)
            for t in (NPR - 2, NPR - 1):
                ex_t = expairs.pop(t)
                nc.tensor.matmul(pv0[0:65, :],
                                 lhsT=V2[:, 2 * t:2 * t + 2, 0, 0:65],
                                 rhs=ex_t[:, :, 0:512], perf_mode=DR,
                                 start=t == 0, stop=t == NPR - 1)
                nc.tensor.matmul(pv1[0:65, :],
                                 lhsT=V2[:, 2 * t:2 * t + 2, 1, 0:65],
                                 rhs=ex_t[:, :, 512:1024], perf_mode=DR,
                                 start=t == 0, stop=t == NPR - 1)
            for fn in [f for s in sorted(nsched) for f in nsched[s]]:
                fn()  # leftovers (shouldn't happen)

            # ---- register the boundary chain into the NEXT q-block ----
            pvc0_t = rpool.tile([65, 512], F32, tag="pvc0", name="pvc0")
            pvc1_t = rpool.tile([65, 512], F32, tag="pvc1", name="pvc1")
            st = {"pvc0": pvc0_t, "pvc1": pvc1_t}

            def s_pvc(st=st, pv0=pv0, pv1=pv1):
                nc.scalar.copy(st["pvc0"], pv0[0:65, :])
                nc.vector.tensor_copy(st["pvc1"], pv1[0:65, :])
                # partition-shift copies: denominators (PSUM row 64) land on
                # partition 0 so the custom-DVE reciprocal can read them
                lden = rpool.tile([1, 1024], F32, tag="lden")
                nc.scalar.copy(lden[0:1, 0:512], pv0[64:65, :])
                nc.scalar.copy(lden[0:1, 512:1024], pv1[64:65, :])
                st["lden"] = lden

            def s_recip(st=st):
                # single-pass custom-DVE approx reciprocal (partition 0 only!)
                rec = rpool.tile([1, 1024], F32, tag="rec")
                nc.vector.reciprocal_approx_fast(rec, st["lden"])
                st["rec"] = rec

            def s_bc(st=st):
                # broadcast 1/l across 64 partitions on the idle GPSIMD
                # engine (replaces the PE outer-product broadcast)
                bcg = rpool.tile([64, 1024], F32, tag="bcg")
                nc.gpsimd.partition_broadcast(bcg, st["rec"][0:1, :])
                st["bcg"] = bcg

            def s_att(st=st):
                bcg = st["bcg"]
                ATT = apool.tile([128, 512], F16, tag="att")
                nc.vector.tensor_mul(ATT[0:64, :], st["pvc0"][0:64, :],
                                     bcg[:, 0:512])
                nc.vector.tensor_mul(ATT[64:128, :], st["pvc1"][0:64, :],
                                     bcg[:, 512:1024])
                st["ATT"] = ATT

            def make_op(qb, st, qs, tail=False):
                def emit():
                    ATT = st["ATT"]
                    # in the tail the score slots are free: rotate the
                    # out-proj PSUM through them so the 4 out-projs pipeline
                    po = (spsum if tail else opsum).tile(
                        [128, 1024], F32, tag="s" if tail else "op2")
                    for ec in range(NEC):
                        mm = nc.tensor.matmul(
                            po[:, ec * 512:(ec + 1) * 512],
                            lhsT=ATT[:, qs * 128:(qs + 1) * 128],
                            rhs=wo_sb[:, ec * 512:(ec + 1) * 512],
                            start=True, stop=True)
                        if fence[0] is not None:
                            add_dep_helper(mm.ins, fence[0].ins, sync=False,
                                           reason="defer out-proj")
                    osb = opool.tile([128, 1024], F16, tag="osb")
                    nc.vector.tensor_copy(osb, po)
                    nc.sync.dma_start(
                        out=out[qb * 512 + qs * 128:
                                qb * 512 + (qs + 1) * 128, :],
                        in_=osb)
                return emit

            if qb + 1 < NSC:
                reg(2, s_pvc)
                reg(4, s_recip)
                reg(6, s_bc)
                reg(9, s_att)
                for qs in range(NQS):
                    reg(12 + 2 * qs, make_op(qb, st, qs))
            else:
                # last q-block: everything runs in the kernel tail
                def s_tail(st=st, pv0=pv0, pv1=pv1):
                    s_pvc()
                    s_recip()
                    s_bc()
                    s_att()
                reg(2, s_tail)
                for qs in range(NQS):
                    reg(4 + qs, make_op(qb, st, qs, tail=True))
            if qb + 2 < NSC:
                reg(10, make_qp(qb + 2))
        # flush the last q-block's chain
        for fn in [f for s in sorted(sched) for f in sched[s]]:
            fn()
    nc.finalize()
    return nc


def _get_nc(S=SEQ, mmdt="fp16"):
    key = (S, mmdt)
    if key not in _NC_CACHE:
        _NC_CACHE[key] = _build_nc(S=S, mmdt=mmdt)
    return _NC_CACHE[key]


def _make_in_maps(x, Wq, bq, Wk, bk, Wv, bv, Wo, npdt=np.float16):
    ET = EMBED // 128
    xT = np.ascontiguousarray(np.asarray(x, np.float32)[0].T.astype(npdt))
    Wq, Wk, Wv, Wo = (np.asarray(a, np.float32).astype(npdt) for a in (Wq, Wk, Wv, Wo))
    bq, bk, bv = (np.asarray(a, np.float32) for a in (bq, bk, bv))

    def wre(W, sl):
        # [E, HC] -> [128, ET*HC] with element (p, a*HC+c) = W[a*128+p, c]
        return np.ascontiguousarray(
            W[:, sl].reshape(ET, 128, HC).transpose(1, 0, 2).reshape(128, ET * HC))

    in_maps = []
    for c in range(N_CORES):
        sl = slice(c * HC, (c + 1) * HC)
        in_maps.append({
            "xT": xT,
            "wq": wre(Wq, sl),
            "wk": wre(Wk, sl),
            "wv": wre(Wv, sl),
            "bq": np.ascontiguousarray(bq[sl]).reshape(HC, 1),
            "bk": np.ascontiguousarray(bk[sl]).reshape(HC, 1),
            "bv": np.ascontiguousarray(bv[sl]).reshape(HC, 1),
            "wo": np.ascontiguousarray(Wo[sl, :]),
        })
    return in_maps


def run(inputs, trace=False, mmdt="fp16"):
    """Run the kernel; returns (out [1,S,E] float32, BassKernelResults)."""
    from concourse.bass_utils import run_bass_kernel_spmd

    nc = _get_nc(mmdt=mmdt)
    in_maps = _make_in_maps(
        inputs["x"], inputs["Wq"], inputs["bq"], inputs["Wk"], inputs["bk"],
        inputs["Wv"], inputs["bv"], inputs["Wo"],
    )
    res = run_bass_kernel_spmd(
        nc, in_maps, core_ids=list(range(N_CORES)), trace=trace
    )
    acc = np.zeros((SEQ, EMBED), np.float64)
    for c in range(N_CORES):
        acc += res.results[c]["out"].astype(np.float64)
    acc += np.asarray(inputs["bo"], np.float64)
    return acc.astype(np.float32).reshape(1, SEQ, EMBED), res


def kernel(x, Wq, bq, Wk, bk, Wv, bv, Wo, bo):
    out, _ = run(dict(x=x, Wq=Wq, bq=bq, Wk=Wk, bk=bk, Wv=Wv, bv=bv, Wo=Wo, bo=bo))
    return out

